# revision 59
# baseline (speedup 1.0000x reference)
"""DualHOILoss Trainium2 kernel (8 NeuronCores, pure data parallel over batch).

Math (per batch b, point p, vert o):
    t_p = (basis_p + delta_p) / s + m           (u = basis + delta, w_o = o - m)
    d2[p,o] = |t_p - o|^2 = u.(-2w/s) + |w|^2 + |u|^2/s^2
computed as ONE K=5 bf16 matmul per 128-point tile: lhsT rows
[ux,uy,uz,1,|u|^2/s^2], rhs rows [-2wx/s,-2wy/s,-2wz/s,|w|^2,1] so PSUM
holds d2 directly.  The host packs the (tiny) coefficient tensors: lhsT in
transposed matmul layout, rhs rows, and the per-point u/|u|^2 slab in
partition layout; the device does all the O(P*V) work.

Vert min (778 verts) per tile: verts split 389 (PSUM bankA) + 389 (PSUM
bankB).  ACT drains bankB pairs (2 tiles per ACT op) to SBUF; one DVE
tensor_tensor_scan per tile folds bankA (PSUM) against the drained copy
(min,min) - 2 streams per DVE cycle, the best min rate on the core.  Scan
tails land in 4 rotating slabs; Pool extracts 4 tails per strided copy.

The selected-anchor distance never goes through the matmul: the host
gathers the selected anchor coords per point (pure indexing); the device
computes d2_sel = |u|^2/s^2 + |w_sel|^2 - (2/s) u.w_sel elementwise on
Pool/DVE, then sqrt on ACT.  Activation tables load exactly twice (sqrt
during the DMA window, exp at batch-0 tail).  Loss partials accumulate via
ACT Square+accum into a [128,4] slab; the host does the final partition
sum.

Point tiling uses the SBUF-natural index map p = 32*q + tau (partition q,
tile tau) so every DMA is contiguous.
"""

import numpy as np

B, P, A, V = 16, 4096, 32, 778
NCORES = 8
BPC = B // NCORES      # batches per core
NT = P // 128          # 32 point tiles per batch
L = 389                # vert cols per PSUM bank (2*L == V)
INF = 3.0e38

_CACHE = {}


def _build_program():
    import concourse.bacc as bacc
    import concourse.mybir as mybir
    from concourse import tile

    f32 = mybir.dt.float32
    bf16 = mybir.dt.bfloat16
    AF = mybir.ActivationFunctionType
    ALU = mybir.AluOpType
    AX = mybir.AxisListType

    nc = bacc.Bacc(None, target_bir_lowering=False)

    # big: per-batch f32 slab [rsel(32) | chc(64) | sbc(12)]
    big_d = nc.dram_tensor("big", [BPC, 128, 108], f32, kind="ExternalInput")
    # ltr: per-batch bf16 slab [rhs rows (V) | lhsT tiles (128*NT)]
    ltr_d = nc.dram_tensor("ltr", [BPC, 5, V + 128 * NT], bf16,
                           kind="ExternalInput")
    out_d = nc.dram_tensor("partials", [128, 5], f32, kind="ExternalOutput")
    tl_d = nc.dram_tensor("tails", [128, 8], f32, kind="ExternalOutput")

    with tile.TileContext(nc) as tc:
        with (
            tc.tile_pool(name="sb", bufs=1) as sb,          # persistent
            tc.tile_pool(name="psA", bufs=3, space="PSUM") as psA,
            tc.tile_pool(name="psB", bufs=2, space="PSUM") as psB,
        ):
            # ---- consolidated DMAs (HWDGE is one serial device: fewer,
            # ---- bigger transfers; lhsT halves so batch 0 starts early)
            bigs, ltrs = [], []
            for b in range(BPC):
                big = sb.tile([128, 108], f32, tag=f"big_{b}", name=f"big_{b}")
                bigs.append(big)
                ltr = sb.tile([5, V + 128 * NT], bf16, tag=f"ltr_{b}",
                              name=f"ltr_{b}")
                ltrs.append(ltr)
            h0 = V + 4 * 128
            hh = V + 64 * NT
            nc.sync.dma_start(ltrs[0][:, 0:h0], ltr_d[0][:, 0:h0])
            nc.sync.dma_start(bigs[0][:], big_d[0])
            nc.sync.dma_start(ltrs[0][:, h0:hh], ltr_d[0][:, h0:hh])
            nc.sync.dma_start(ltrs[0][:, hh:], ltr_d[0][:, hh:])
            nc.sync.dma_start(bigs[1][:], big_d[1])
            nc.sync.dma_start(ltrs[1][:, 0:hh], ltr_d[1][:, 0:hh])
            nc.sync.dma_start(ltrs[1][:, hh:], ltr_d[1][:, hh:])
            lts = [ltrs[b][:, V : V + 128 * NT] for b in range(BPC)]
            rhss = [ltrs[b][:, 0:V] for b in range(BPC)]
            rsels = [bigs[b][:, 0:NT] for b in range(BPC)]
            chcs = [bigs[b][:, NT : 3 * NT] for b in range(BPC)]

            part = sb.tile([128, 5], f32, tag="part")
            nc.gpsimd.memset(part[:], 0.0)

            # PE p-state warmup: chain of dummy matmuls so the real ones hit
            # full clock (ramp needs ~3us of continuous PE busy)
            wtile = sb.tile([5, 512], bf16, tag="wtile")
            nc.gpsimd.memset(wtile[:], 0.0)
            wps = psB.tile([128, 1024], f32, tag="ptB")
            for _ in range(3):
                nc.tensor.matmul(wps[:, 0:512], wtile[:, 0:128], wtile[:],
                                 start=True, stop=True)
            # dummy activations on constant data pull both table loads into
            # the pre-loop ACT-idle window (real sqrt/exp are then load-free)
            dume = sb.tile([5, 16], f32, tag="dume")
            nc.scalar.activation(dume[:], wtile[:, 0:16], AF.Sqrt)

            # 4 drain buffers; per-batch scan-tail slabs (no extracts: the
            # contact exp reads the 32 tails through a strided AP)
            c2bufs, junkbigs = [], []
            for i in range(8):
                c2b = sb.tile([128, 2 * L], f32, tag=f"c2_{i}", name=f"c2_{i}")
                c2bufs.append(c2b)
            for b in range(BPC):
                jbt = sb.tile([128, NT * L], f32, tag=f"jkb_{b}",
                              name=f"jkb_{b}")
                junkbigs.append(jbt)

            dsels = []

            # choir/contact finishers, interleaved into the tile loops at
            # points where ACT has accumulated slack over DVE
            def choir_sqrt(b):
                dsel = sb.tile([128, NT], f32, tag="dsel", bufs=2,
                               name=f"dsel_{b}")
                nc.scalar.activation(dsel[:], rsels[b], AF.Sqrt)
                dsels.append(dsel)

            def choir_fin(b):
                chv = chcs[b].rearrange("p (t s) -> p t s", s=2)
                ddiff = sb.tile([128, NT], f32, tag="ddiff", bufs=2,
                                name=f"ddiff_{b}")
                nc.gpsimd.tensor_tensor(
                    ddiff[:], dsels[b][:], chv[:, :, 0:1].squeeze(2),
                    op=ALU.subtract)
                jnk = sb.tile([128, NT], f32, tag="jnkd", bufs=2)
                nc.scalar.activation(jnk[:], ddiff[:], AF.Square,
                                     accum_out=part[:, b : b + 1])

            def contact_fin(b, t0=0, t1=NT, slot=None):
                w = t1 - t0
                chv = chcs[b].rearrange("p (t s) -> p t s", s=2)
                tails = junkbigs[b][:].rearrange(
                    "p (t w) -> p t w", w=L)[:, t0:t1, L - 1 : L].squeeze(2)
                cont = sb.tile([128, w], f32, tag="cont", bufs=2)
                nc.scalar.activation(cont[:], tails, AF.Exp, scale=-100.0)
                cdiff = sb.tile([128, w], f32, tag="cdiff", bufs=2)
                nc.gpsimd.tensor_tensor(
                    cdiff[:], cont[:], chv[:, t0:t1, 1:2].squeeze(2),
                    op=ALU.subtract)
                jnk2 = sb.tile([128, w], f32, tag="jnkc", bufs=2)
                c = 2 + b if slot is None else slot
                nc.scalar.activation(jnk2[:], cdiff[:], AF.Square,
                                     accum_out=part[:, c : c + 1])

            dume2 = sb.tile([5, 16], f32, tag="dume2")

            def exp_preload():
                nc.scalar.activation(dume2[:], dsels[1][0:5, 0:16], AF.Exp)

            hooks = {
                (0, 6): lambda: choir_sqrt(0),
                (0, 9): lambda: choir_fin(0),
                (0, 12): lambda: choir_sqrt(1),
                (0, 15): lambda: (choir_fin(1), exp_preload()),
                (1, 2): lambda: contact_fin(0),
                (1, 12): lambda: contact_fin(1, 0, 24, 3),
                (1, 14): lambda: nc.sync.dma_start(out_d[:], part[:]),
            }

            # ---------------- tile loops ----------------
            for b in range(BPC):
                lt = lts[b]
                rhs = rhss[b]
                jbt = junkbigs[b]
                for kp in range(NT // 2):
                    c2 = c2bufs[kp % 8]
                    c2v = c2[:].rearrange("p (j w) -> p j w", j=2)
                    ptB = psB.tile([128, 1024], f32, tag="ptB")
                    for j in range(2):
                        t = 2 * kp + j
                        nc.tensor.matmul(ptB[:, 512 * j : 512 * j + L],
                                         lt[:, 128 * t : 128 * (t + 1)],
                                         rhs[:, L:V], start=True, stop=True)
                    ptAs = []
                    for j in range(2):
                        t = 2 * kp + j
                        ptA = psA.tile([128, 512], f32, tag="ptA")
                        ptAs.append(ptA)
                        nc.tensor.matmul(ptA[:, 0:L],
                                         lt[:, 128 * t : 128 * (t + 1)],
                                         rhs[:, 0:L], start=True, stop=True)
                    nc.scalar.activation(
                        c2v[:, :, :],
                        ptB[:].rearrange("p (j w) -> p j w", j=2)[:, :, 0:L],
                        AF.Copy,
                    )
                    for j in range(2):
                        t = 2 * kp + j
                        nc.vector.tensor_tensor_scan(
                            out=jbt[:, L * t : L * (t + 1)],
                            data0=ptAs[j][:, 0:L], data1=c2v[:, j, :],
                            initial=INF, op0=ALU.min, op1=ALU.min)
                    hook = hooks.get((b, kp))
                    if hook is not None:
                        hook()
                contact_fin_b = b

            # last 8 tiles' min-d2 tails go to the host raw: the exp/square
            # tail math would serialize after the final scan
            nc.sync.dma_start(
                tl_d[:],
                junkbigs[1][:].rearrange(
                    "p (t w) -> p t w", w=L)[:, 24:NT, L - 1 : L].squeeze(2))

    nc.compile()
    return nc


def _get_program():
    if "nc" not in _CACHE:
        _CACHE["nc"] = _build_program()
    return _CACHE["nc"]


def _pack(verts, anchors, choir, hand_contacts, bps_mean, bps_scalar,
          bps_basis):
    """Host-side layout packing of the small coefficient tensors."""
    import ml_dtypes
    verts = np.ascontiguousarray(np.asarray(verts, np.float32))
    anchors = np.ascontiguousarray(np.asarray(anchors, np.float32))
    choir = np.ascontiguousarray(np.asarray(choir, np.float32))
    hand_contacts = np.ascontiguousarray(np.asarray(hand_contacts, np.float32))
    bps_mean = np.asarray(bps_mean, np.float32).reshape(B, 3)
    s = np.float32(np.asarray(bps_scalar).reshape(()))
    basis = np.asarray(bps_basis, np.float32).reshape(P, 3)

    # per-point target slab [anc_d, hc] with p = 32q + tau map
    chc = np.concatenate(
        [choir[:, :, 4:5], hand_contacts[:, :, None]], axis=2,
    ).reshape(B, 128, 2 * NT)
    idx = choir[:, :, 5].astype(np.int64)
    asel = np.take_along_axis(anchors, idx[:, :, None], axis=1)  # (B,P,3)

    u = basis[None] + choir[:, :, 1:4]                       # (B,P,3)
    uu2 = (u * u).sum(-1) / (s * s)                          # (B,P)
    # lhsT layout: ut5[b, r, 128*t + q] = row r of point p = 32*q + t
    ur = u.reshape(B, 128, NT, 3)
    ut5 = np.empty((B, 5, NT, 128), np.float32)
    ut5[:, 0:3] = ur.transpose(0, 3, 2, 1)
    ut5[:, 3] = 1.0
    ut5[:, 4] = uu2.reshape(B, 128, NT).transpose(0, 2, 1)
    ut5 = ut5.reshape(B, 5, 128 * NT)
    # rhs rows [-2w/s (3), |w|^2, 1]
    w = verts - bps_mean[:, None, :]                         # (B,V,3)
    rhs5 = np.empty((B, 5, V), np.float32)
    rhs5[:, 0:3] = (w * (np.float32(-2.0) / s)).transpose(0, 2, 1)
    rhs5[:, 3] = (w * w).sum(-1)
    rhs5[:, 4] = 1.0

    # selected-anchor squared distance (clamped), ready for device sqrt
    wselc = asel - bps_mean[:, None, :]
    q = (wselc * wselc).sum(-1) - (np.float32(2.0) / s) * (u * wselc).sum(-1)
    rsel = np.maximum(q + uu2, np.float32(1.0e-12)).reshape(B, 128, NT)

    # consolidated slabs
    ltr = np.concatenate([rhs5, ut5], axis=2).astype(ml_dtypes.bfloat16)
    big = np.zeros((B, 128, 108), np.float32)
    big[:, :, 0:NT] = rsel
    big[:, :, NT : 3 * NT] = chc

    in_maps = []
    for c in range(NCORES):
        lo = BPC * c
        bigc = big[lo : lo + BPC].copy()
        in_maps.append({
            "big": bigc,
            "ltr": ltr[lo : lo + BPC],
        })
    return in_maps


def kernel(verts, anchors, choir, hand_contacts, bps_mean, bps_scalar,
           bps_basis, _trace=False):
    from concourse.bass_utils import run_bass_kernel_spmd

    nc = _get_program()
    in_maps = _pack(verts, anchors, choir, hand_contacts, bps_mean,
                    bps_scalar, bps_basis)
    res = run_bass_kernel_spmd(nc, in_maps, list(range(NCORES)))
    parts = np.stack([np.asarray(r["partials"], np.float64).reshape(128, 5)
                      for r in res.results])
    psum = parts.sum(axis=(0, 1))
    hc = np.asarray(hand_contacts, np.float32).reshape(B, 128, NT)
    tail_sum = 0.0
    for c in range(NCORES):
        tails = np.asarray(res.results[c]["tails"], np.float64)  # (128, 8)
        hcs = hc[BPC * c + 1, :, 24:NT].astype(np.float64)
        tail_sum += ((hcs - np.exp(-100.0 * tails)) ** 2).sum()
    choir_loss = (psum[0] + psum[1]) / (B * P)
    contact_loss = (psum[2] + psum[3] + tail_sum) / (B * P)
    out = (np.float32(choir_loss), np.float32(contact_loss))
    if _trace:
        return out, res
    return out
